# revision 1
# baseline (speedup 1.0000x reference)
"""Tensor-sketch (CountSketch2) kernel for 8 Trainium2 NeuronCores.

out = Im(ifft(fft(xcs + 1j*ycs)**2)) / 2   where xcs/ycs are count sketches
of x/y into M=8192 buckets (the imaginary part of the squared packed FFT is
exactly 2*(xcs (*) ycs), the circular convolution — so one forward transform,
one square, one inverse transform).

FFT of length M = 8192 is factored as 128(b1) x 64(b2), b = b1*64 + b2,
f = f1 + 128*f2, and executed as dense einsums (twiddles folded into the
per-b2 stage-1 matrices).  The count-sketch scatter runs on device as a
one-hot matmul x @ S (S[i, indx[i]] = sign[i]).  Everything is data-parallel
over the batch dim across the 8 cores via shard_map.
"""
import numpy as np
import jax
import jax.numpy as jnp
from jax.sharding import Mesh, PartitionSpec as P
from jax.experimental.shard_map import shard_map
from functools import partial

N = 4096
D = 4096
M = 8192
M1 = 128
M2 = 64
NCORES = 8

_cache = {}


def _stationaries():
    if "A" in _cache:
        return _cache
    b1 = np.arange(M1)
    f1 = np.arange(M1)
    b2 = np.arange(M2)
    W = np.exp(-2j * np.pi / M)
    # A[b1, f1, b2] = W^(f1*(64*b1 + b2))  (stage-1 DFT-128 with twiddle folded)
    A = W ** (f1[None, :, None] * (M2 * b1[:, None, None] + b2[None, None, :]))
    W64 = np.exp(-2j * np.pi * np.outer(b2, b2) / M2)          # [b2, f2]
    _cache["Ar"] = A.real.astype(np.float32)
    _cache["Ai"] = A.imag.astype(np.float32)
    _cache["Kr"] = W64.real.astype(np.float32)
    _cache["Ki"] = W64.imag.astype(np.float32)
    _cache["A"] = True
    return _cache


def _core_fn(xs, ys, s1m, s2m, Ar, Ai, Kr, Ki):
    # xs [R, D] f32 local shard; s1m/s2m [D, M] one-hot (sign-valued)
    R = xs.shape[0]
    zre = jnp.dot(xs, s1m)                    # [R, M]  = xcs
    zim = jnp.dot(ys, s2m)                    # [R, M]  = ycs
    vr = zre.reshape(R, M1, M2)
    vi = zim.reshape(R, M1, M2)
    # stage 1 + twiddle:  T[n,f1,b2] = sum_b1 V[n,b1,b2] * A[b1,f1,b2]
    tr = jnp.einsum('nbr,bfr->nfr', vr, Ar) - jnp.einsum('nbr,bfr->nfr', vi, Ai)
    ti = jnp.einsum('nbr,bfr->nfr', vr, Ai) + jnp.einsum('nbr,bfr->nfr', vi, Ar)
    # stage 2:  X[n,f1,f2] = sum_b2 T[n,f1,b2] * W64[b2,f2]
    xr = jnp.einsum('nfr,rg->nfg', tr, Kr) - jnp.einsum('nfr,rg->nfg', ti, Ki)
    xi = jnp.einsum('nfr,rg->nfg', tr, Ki) + jnp.einsum('nfr,rg->nfg', ti, Kr)
    # square
    qr = (xr + xi) * (xr - xi)
    qi = 2.0 * xr * xi
    # inverse stage 2:  U = Q @ conj(W64)
    ur = jnp.einsum('nfg,gr->nfr', qr, Kr) + jnp.einsum('nfg,gr->nfr', qi, Ki)
    ui = jnp.einsum('nfg,gr->nfr', qi, Kr) - jnp.einsum('nfg,gr->nfr', qr, Ki)
    # inverse stage 1 (imag part only):  o = conj(A) . U ; Im(o) = Ar*Ui - Ai*Ur...
    # o[n,b1,b2] = sum_f1 conj(A[b1,f1,b2]) * U[n,f1,b2]
    oi = jnp.einsum('nfr,bfr->nbr', ui, Ar) - jnp.einsum('nfr,bfr->nbr', ur, Ai)
    out = oi.reshape(R, M) * (0.5 / M)
    return out.astype(jnp.float32)


def _build(R):
    st = _stationaries()
    devices = jax.devices()[:NCORES]
    mesh = Mesh(np.asarray(devices), ("core",))
    fn = shard_map(
        _core_fn, mesh=mesh,
        in_specs=(P("core"), P("core"), P(), P(), P(), P(), P(), P()),
        out_specs=P("core"))
    jfn = jax.jit(fn)
    return mesh, jfn


def kernel(x, y, sign1, indx1, sign2, indx2):
    x = np.asarray(x, np.float32)
    y = np.asarray(y, np.float32)
    st = _stationaries()
    s1m = np.zeros((D, M), np.float32)
    s1m[np.arange(D), np.asarray(indx1)] = np.asarray(sign1, np.float32)
    s2m = np.zeros((D, M), np.float32)
    s2m[np.arange(D), np.asarray(indx2)] = np.asarray(sign2, np.float32)
    key = ("jfn", N // NCORES)
    if key not in _cache:
        _cache[key] = _build(N // NCORES)
    mesh, jfn = _cache[key]
    out = jfn(x, y, s1m, s2m, st["Ar"], st["Ai"], st["Kr"], st["Ki"])
    return np.asarray(out, np.float32)


if __name__ == "__main__":
    rng = np.random.default_rng(0)
    x = rng.standard_normal((N, D)).astype(np.float32)
    y = rng.standard_normal((N, D)).astype(np.float32)
    s1 = (rng.integers(0, 2, D) * 2 - 1).astype(np.float32)
    s2 = (rng.integers(0, 2, D) * 2 - 1).astype(np.float32)
    i1 = rng.integers(0, M, D).astype(np.int32)
    i2 = rng.integers(0, M, D).astype(np.int32)
    o = kernel(x, y, s1, i1, s2, i2)
    print("kernel ok", o.shape, o.dtype, float(np.abs(o).max()))

